# revision 7
# baseline (speedup 1.0000x reference)
"""Multi-head attention (B=2, S=2048, D=1024, H=16) on 8 Trainium2 NeuronCores.

Sharding: tensor-parallel over heads — 2 heads per core. Each core computes
its heads' QKV projection, attention, and a partial FC output (row-slice of
the FC contraction); the host sums the 8 partials and adds the FC bias.

Per-core pipeline (all matmuls in float32r — full-rate TF32-class):
  1. QKV projection: qT/kT [128f, 4096t] transposed layouts, vT likewise.
     Score scale 1/8 and biases folded into PSUM eviction.
  2. Per (batch, head): V re-transposed to key-major [keys, 64] via PE.
  3. ScoresT [keys, q] = K^T Q per 128-key tile; exp on ACT (scores are
     bounded ~[-3, 4.5] so unsafe softmax is exact); AV accumulates
     values^T [64, q] and the softmax denominators via a parallel
     ones-matmul in the other PE column strip.
  4. Normalization: reciprocal of denominators, partition-move via DMA,
     elementwise multiply into valuesT [128f, 4096t].
  5. FC: partial[t, e] = valuesT[:, t]^T @ w_fc^T slice; DMA to DRAM.
"""
import numpy as np
from contextlib import ExitStack

import concourse.bass as bass
import concourse.tile as tile
from concourse import bacc, mybir
from concourse.bass_utils import run_bass_kernel_spmd
from concourse.masks import make_identity

B, S, D, H, HD = 2, 2048, 1024, 16, 64
T = B * S                # 4096 tokens
NC = 8                   # cores
HPC = H // NC            # heads per core
F = HPC * HD             # 128 value-features per core
KT = 128                 # key tile (contraction tile for AV)
QB = 512                 # query block (matmul free dim)
f32 = mybir.dt.float32
f32r = mybir.dt.float32r
AF = mybir.ActivationFunctionType
OP = mybir.AluOpType

_NC_CACHE = None


def _build():
    nc = bacc.Bacc("TRN2", target_bir_lowering=False, debug=False, num_devices=NC)

    XT = nc.dram_tensor("xT", [D, T], f32r, kind="ExternalInput").ap()
    WQ = nc.dram_tensor("wq", [D, F], f32r, kind="ExternalInput").ap()
    WK = nc.dram_tensor("wk", [D, F], f32r, kind="ExternalInput").ap()
    WV = nc.dram_tensor("wv", [D, F], f32r, kind="ExternalInput").ap()
    BQ = nc.dram_tensor("bq", [F, 1], f32, kind="ExternalInput").ap()
    BK = nc.dram_tensor("bk", [F, 1], f32, kind="ExternalInput").ap()
    BV = nc.dram_tensor("bv", [F, 1], f32, kind="ExternalInput").ap()
    WFC = nc.dram_tensor("wfc", [F, D], f32r, kind="ExternalInput").ap()
    OUT = nc.dram_tensor("out", [T, D], f32, kind="ExternalOutput").ap()

    with tile.TileContext(nc) as tc, ExitStack() as ctx:
        const = ctx.enter_context(tc.tile_pool(name="const", bufs=1))
        xt_pool = ctx.enter_context(tc.tile_pool(name="xt", bufs=12))
        big = ctx.enter_context(tc.tile_pool(name="big", bufs=1))
        vt_pool = ctx.enter_context(tc.tile_pool(name="vt", bufs=2))
        exp_pool = ctx.enter_context(tc.tile_pool(name="expt", bufs=4))
        r_pool = ctx.enter_context(tc.tile_pool(name="recip", bufs=2))
        r2_pool = ctx.enter_context(tc.tile_pool(name="recip2", bufs=2))
        fout_pool = ctx.enter_context(tc.tile_pool(name="fout", bufs=4))

        mm_ps = ctx.enter_context(tc.tile_pool(name="mm_ps", bufs=2, space="PSUM"))
        sc_ps = ctx.enter_context(tc.tile_pool(name="sc_ps", bufs=2, space="PSUM"))
        av_ps = ctx.enter_context(tc.tile_pool(name="av_ps", bufs=2, space="PSUM"))
        tr_ps = ctx.enter_context(tc.tile_pool(name="tr_ps", bufs=2, space="PSUM"))

        # --- constants ---
        wq_sb = const.tile([128, D // 128, F], f32r)
        nc.sync.dma_start(out=wq_sb, in_=WQ.rearrange("(t p) f -> p t f", p=128))
        wk_sb = const.tile([128, D // 128, F], f32r)
        nc.sync.dma_start(out=wk_sb, in_=WK.rearrange("(t p) f -> p t f", p=128))
        wv_sb = const.tile([128, D // 128, F], f32r)
        nc.sync.dma_start(out=wv_sb, in_=WV.rearrange("(t p) f -> p t f", p=128))
        wfc_sb = const.tile([F, D], f32r)
        nc.sync.dma_start(out=wfc_sb, in_=WFC)
        bq_sb = const.tile([F, 1], f32)
        nc.sync.dma_start(out=bq_sb, in_=BQ)
        bk_sb = const.tile([F, 1], f32)
        nc.sync.dma_start(out=bk_sb, in_=BK)
        bv_sb = const.tile([F, 1], f32)
        nc.sync.dma_start(out=bv_sb, in_=BV)

        ident = const.tile([128, 64], f32)  # I_64 stacked in both halves
        make_identity(nc, ident[0:64, :])
        make_identity(nc, ident[64:128, :])
        ones_f = const.tile([128, S // KT * HD], f32)
        nc.vector.memset(ones_f, 1.0)

        qT = big.tile([128, T], f32r)   # [Qh0(64) ; Qh1(64)] x tokens, pre-scaled 1/8
        kT = big.tile([128, T], f32r)
        vT = big.tile([128, T], f32)
        valuesT = big.tile([128, T], f32r)

        # --- phase 1: QKV projection (transposed outputs) ---
        for tb in range(T // QB):
            xts = []
            for kt in range(D // 128):
                xt = xt_pool.tile([128, QB], f32r, tag="xt")
                nc.sync.dma_start(
                    out=xt, in_=XT[kt * 128:(kt + 1) * 128, tb * QB:(tb + 1) * QB]
                )
                xts.append(xt)
            for w_sb, dst, bias_ap, scale in (
                (wq_sb, qT, bq_sb, 0.125),
                (wk_sb, kT, bk_sb, None),
                (wv_sb, vT, bv_sb, None),
            ):
                ps = mm_ps.tile([128, QB], f32, tag="mm512")
                for kt in range(D // 128):
                    nc.tensor.matmul(
                        ps, w_sb[:, kt, :], xts[kt],
                        start=(kt == 0), stop=(kt == D // 128 - 1),
                    )
                dslice = dst[:, tb * QB:(tb + 1) * QB]
                if scale is None:
                    nc.vector.tensor_scalar_add(dslice, ps, bias_ap)
                else:
                    nc.vector.tensor_scalar(
                        dslice, ps, bias_ap, scale, op0=OP.add, op1=OP.mult
                    )

        # --- phases 2-5 per batch ---
        for b in range(B):
            t0 = b * S
            # V re-transposed to key-major + ones block for the denominators:
            # head h's lhsT tile [128 keys, 128] has V in cols hp:hp+64 (so
            # values land in psum partitions hp:hp+64) and ones in the rest.
            vkm = []  # per head: [128, S//KT, 128]
            for h in range(HPC):
                hp, op_ = h * HD, (1 - h) * HD
                vk = vt_pool.tile([128, S // KT, 128], f32r, tag=f"vk{h}")
                nc.vector.tensor_copy(vk[:, :, op_:op_ + HD], ones_f)
                for kt in range(S // KT):
                    tp = tr_ps.tile([128, HD], f32, tag="tr")
                    nc.tensor.transpose(
                        tp,
                        vT[h * HD:(h + 1) * HD, t0 + kt * KT: t0 + (kt + 1) * KT],
                        ident[h * HD:(h + 1) * HD, :],
                    )
                    nc.vector.tensor_copy(vk[:, kt, hp:hp + HD], tp)
                vkm.append(vk)

            for qb in range(S // QB):
                q0 = t0 + qb * QB
                for h in range(HPC):
                    hp = h * HD          # partition base of this head's rows
                    op_ = (1 - h) * HD   # the other 64-partition half
                    pav = av_ps.tile([128, QB], f32, tag="av")
                    for kt in range(S // KT):
                        k0 = t0 + kt * KT
                        sc = sc_ps.tile([128, QB], f32, tag="sc")
                        nc.tensor.matmul(
                            sc,
                            kT[hp:hp + HD, k0:k0 + KT],
                            qT[hp:hp + HD, q0:q0 + QB],
                            start=True, stop=True,
                            tile_position=(hp, 0),
                        )
                        et = exp_pool.tile([128, QB], f32r, tag="expt")
                        nc.scalar.activation(et, sc, AF.Exp)
                        first, last = kt == 0, kt == S // KT - 1
                        # [V|ones] lhsT: values^T into partitions hp:hp+64,
                        # softmax denominators into the other 64 partitions
                        nc.tensor.matmul(
                            pav, vkm[h][:, kt, :], et,
                            start=first, stop=last,
                        )
                    rec = r_pool.tile([128, QB], f32, tag="rec")
                    nc.vector.reciprocal(rec[op_:op_ + HD, :], pav[op_:op_ + HD, :])
                    rec2 = r2_pool.tile([128, QB], f32, tag="rec2")
                    nc.sync.dma_start(
                        out=rec2[hp:hp + HD, :], in_=rec[op_:op_ + HD, :]
                    )
                    nc.vector.tensor_mul(
                        valuesT[hp:hp + HD, q0:q0 + QB],
                        pav[hp:hp + HD, :],
                        rec2[hp:hp + HD, :],
                    )

            # FC partial for this batch's tokens
            for tb2 in range(S // 128):
                tt = t0 + tb2 * 128
                for eb in range(D // QB):
                    fp = mm_ps.tile([128, QB], f32, tag="mm512")
                    nc.tensor.matmul(
                        fp, valuesT[:, tt:tt + 128],
                        wfc_sb[:, eb * QB:(eb + 1) * QB],
                        start=True, stop=True,
                    )
                    fo = fout_pool.tile([128, QB], f32, tag="fout")
                    nc.vector.tensor_copy(fo, fp)
                    nc.sync.dma_start(
                        out=OUT[tt:tt + 128, eb * QB:(eb + 1) * QB], in_=fo
                    )

    nc.compile()
    return nc


def _get_nc():
    global _NC_CACHE
    if _NC_CACHE is None:
        _NC_CACHE = _build()
    return _NC_CACHE


def _prep_in_maps(x, w_qkv, b_qkv, w_fc):
    xT = np.ascontiguousarray(x.reshape(T, D).T).astype(np.float32)
    in_maps = []
    for c in range(NC):
        heads = [HPC * c + i for i in range(HPC)]
        rows = {
            "q": np.concatenate([np.arange(h * 3 * HD, h * 3 * HD + HD) for h in heads]),
            "k": np.concatenate([np.arange(h * 3 * HD + HD, h * 3 * HD + 2 * HD) for h in heads]),
            "v": np.concatenate([np.arange(h * 3 * HD + 2 * HD, h * 3 * HD + 3 * HD) for h in heads]),
        }
        m = {
            "xT": xT,
            "wq": np.ascontiguousarray(w_qkv[rows["q"]].T),
            "wk": np.ascontiguousarray(w_qkv[rows["k"]].T),
            "wv": np.ascontiguousarray(w_qkv[rows["v"]].T),
            "bq": np.ascontiguousarray(b_qkv[rows["q"]][:, None]),
            "bk": np.ascontiguousarray(b_qkv[rows["k"]][:, None]),
            "bv": np.ascontiguousarray(b_qkv[rows["v"]][:, None]),
            "wfc": np.ascontiguousarray(w_fc[:, c * F:(c + 1) * F].T),
        }
        in_maps.append(m)
    return in_maps


def run_kernel(inputs, trace=False, trace_cores=None):
    x = np.asarray(inputs["x"], np.float32)
    w_qkv = np.asarray(inputs["w_qkv"], np.float32)
    b_qkv = np.asarray(inputs["b_qkv"], np.float32)
    w_fc = np.asarray(inputs["w_fc"], np.float32)
    b_fc = np.asarray(inputs["b_fc"], np.float32)

    nc = _get_nc()
    in_maps = _prep_in_maps(x, w_qkv, b_qkv, w_fc)
    res = run_bass_kernel_spmd(
        nc, in_maps, core_ids=list(range(NC)), trace=trace,
        trace_cores=trace_cores,
    )
    out = res.results[0]["out"].astype(np.float32)
    for r in res.results[1:]:
        out = out + r["out"]
    out = out + b_fc[None, :]
    return out.reshape(B, S, D), res


def kernel(**inputs):
    out, _ = run_kernel(inputs, trace=False)
    return out


# revision 10
# speedup vs baseline: 1.3255x; 1.3255x over previous
"""Multi-head attention (B=2, S=2048, D=1024, H=16) on 8 Trainium2 NeuronCores.

Sharding: tensor-parallel over heads — 2 heads per core. Each core computes
its heads' QKV projection, attention, and a partial FC output (row-slice of
the FC contraction); the host sums the 8 partials and adds the FC bias.

Per-core pipeline (all matmuls in float32r — full-rate TF32-class):
  1. QKV projection: qT/kT [128f, 4096t] transposed layouts, vT likewise.
     Score scale 1/8 and biases folded into PSUM eviction.
  2. Per (batch, head): V re-transposed to key-major [keys, 64] via PE.
  3. ScoresT [keys, q] = K^T Q per 128-key tile; exp on ACT (scores are
     bounded ~[-3, 4.5] so unsafe softmax is exact); AV accumulates
     values^T [64, q] and the softmax denominators via a parallel
     ones-matmul in the other PE column strip.
  4. Normalization: reciprocal of denominators, partition-move via DMA,
     elementwise multiply into valuesT [128f, 4096t].
  5. FC: partial[t, e] = valuesT[:, t]^T @ w_fc^T slice; DMA to DRAM.
"""
import numpy as np
from contextlib import ExitStack

import concourse.bass as bass
import concourse.tile as tile
from concourse import bacc, mybir
from concourse.bass_utils import run_bass_kernel_spmd
from concourse.masks import make_identity

B, S, D, H, HD = 2, 2048, 1024, 16, 64
T = B * S                # 4096 tokens
NC = 8                   # cores
HPC = H // NC            # heads per core
F = HPC * HD             # 128 value-features per core
KT = 128                 # key tile (contraction tile for AV)
QB = 512                 # query block (matmul free dim)
f32 = mybir.dt.float32
f32r = mybir.dt.float32r
AF = mybir.ActivationFunctionType
OP = mybir.AluOpType

_NC_CACHE = None


def _build():
    nc = bacc.Bacc("TRN2", target_bir_lowering=False, debug=False, num_devices=NC)

    XT = nc.dram_tensor("xT", [D, T], f32r, kind="ExternalInput").ap()
    WQ = nc.dram_tensor("wq", [D, F], f32r, kind="ExternalInput").ap()
    WK = nc.dram_tensor("wk", [D, F], f32r, kind="ExternalInput").ap()
    WV = nc.dram_tensor("wv", [D, F], f32r, kind="ExternalInput").ap()
    BQ = nc.dram_tensor("bq", [F, 1], f32, kind="ExternalInput").ap()
    BK = nc.dram_tensor("bk", [F, 1], f32, kind="ExternalInput").ap()
    BV = nc.dram_tensor("bv", [F, 1], f32, kind="ExternalInput").ap()
    WFC = nc.dram_tensor("wfc", [F, D], f32r, kind="ExternalInput").ap()
    OUT = nc.dram_tensor("out", [T, D], f32, kind="ExternalOutput").ap()

    with tile.TileContext(nc) as tc, ExitStack() as ctx:
        const = ctx.enter_context(tc.tile_pool(name="const", bufs=1))
        xt_pool = ctx.enter_context(tc.tile_pool(name="xt", bufs=12))
        big = ctx.enter_context(tc.tile_pool(name="big", bufs=1))
        vt_pool = ctx.enter_context(tc.tile_pool(name="vt", bufs=2))
        exp_pool = ctx.enter_context(tc.tile_pool(name="expt", bufs=4))
        r_pool = ctx.enter_context(tc.tile_pool(name="recip", bufs=2))
        r2_pool = ctx.enter_context(tc.tile_pool(name="recip2", bufs=2))
        fout_pool = ctx.enter_context(tc.tile_pool(name="fout", bufs=4))

        mm_ps = ctx.enter_context(tc.tile_pool(name="mm_ps", bufs=2, space="PSUM"))
        sc_ps = ctx.enter_context(tc.tile_pool(name="sc_ps", bufs=2, space="PSUM"))
        av_ps = ctx.enter_context(tc.tile_pool(name="av_ps", bufs=1, space="PSUM"))

        # --- constants ---
        wq_sb = const.tile([128, D // 128, F], f32r)
        nc.sync.dma_start(out=wq_sb, in_=WQ.rearrange("(t p) f -> p t f", p=128))
        wk_sb = const.tile([128, D // 128, F], f32r)
        nc.sync.dma_start(out=wk_sb, in_=WK.rearrange("(t p) f -> p t f", p=128))
        wv_sb = const.tile([128, D // 128, F], f32r)
        nc.sync.dma_start(out=wv_sb, in_=WV.rearrange("(t p) f -> p t f", p=128))
        wfc_sb = const.tile([F, D], f32r)
        nc.sync.dma_start(out=wfc_sb, in_=WFC)
        bq_sb = const.tile([F, 1], f32)
        nc.sync.dma_start(out=bq_sb, in_=BQ)
        bk_sb = const.tile([F, 1], f32)
        nc.sync.dma_start(out=bk_sb, in_=BK)
        bv_sb = const.tile([F, 1], f32)
        nc.sync.dma_start(out=bv_sb, in_=BV)

        ident = const.tile([128, 64], f32)  # I_64 stacked in both halves
        make_identity(nc, ident[0:64, :])
        make_identity(nc, ident[64:128, :])
        ones_f = const.tile([128, S // KT * HD], f32)
        nc.vector.memset(ones_f, 1.0)

        qT = big.tile([128, T], f32r)   # [Qh0(64) ; Qh1(64)] x tokens, pre-scaled 1/8
        kT = big.tile([128, T], f32r)
        vT = big.tile([128, T], f32)
        valuesT = big.tile([128, T], f32r)

        # --- phase 1: QKV projection (transposed outputs) ---
        for tb in range(T // QB):
            xts = []
            for kt in range(D // 128):
                xt = xt_pool.tile([128, QB], f32r, tag="xt")
                nc.sync.dma_start(
                    out=xt, in_=XT[kt * 128:(kt + 1) * 128, tb * QB:(tb + 1) * QB]
                )
                xts.append(xt)
            for w_sb, dst, bias_ap, scale in (
                (wq_sb, qT, bq_sb, 0.125),
                (wk_sb, kT, bk_sb, None),
                (wv_sb, vT, bv_sb, None),
            ):
                ps = mm_ps.tile([128, QB], f32, tag="mm512")
                for kt in range(D // 128):
                    nc.tensor.matmul(
                        ps, w_sb[:, kt, :], xts[kt],
                        start=(kt == 0), stop=(kt == D // 128 - 1),
                    )
                dslice = dst[:, tb * QB:(tb + 1) * QB]
                if scale is None:
                    nc.vector.tensor_scalar_add(dslice, ps, bias_ap)
                else:
                    nc.vector.tensor_scalar(
                        dslice, ps, bias_ap, scale, op0=OP.add, op1=OP.mult
                    )

        # --- phases 2-5 per batch ---
        for b in range(B):
            t0 = b * S
            # V re-transposed to key-major + ones block for the denominators:
            # head h's lhsT tile [128 keys, 128] has V in cols hp:hp+64 (so
            # values land in psum partitions hp:hp+64) and ones in the rest.
            vkm = []  # per head: [128, S//KT, 128]
            for h in range(HPC):
                hp, op_ = h * HD, (1 - h) * HD
                vk = vt_pool.tile([128, S // KT, 128], f32r, tag=f"vk{h}")
                nc.vector.tensor_copy(vk[:, :, op_:op_ + HD], ones_f)
                for kt in range(S // KT):
                    tp = mm_ps.tile([128, HD], f32, tag="mm512")
                    nc.tensor.transpose(
                        tp,
                        vT[h * HD:(h + 1) * HD, t0 + kt * KT: t0 + (kt + 1) * KT],
                        ident[h * HD:(h + 1) * HD, :],
                    )
                    nc.vector.tensor_copy(vk[:, kt, hp:hp + HD], tp)
                vkm.append(vk)

            for qb in range(S // QB):
                q0 = t0 + qb * QB
                # both heads share one [128, 2*QB] score tile so exp runs as
                # a single wide ACTIVATE; the heads' score matmuls sit in
                # disjoint PE row strips and run concurrently.
                pav = [av_ps.tile([128, QB], f32, tag=f"pav{h}",
                                  name=f"pav{h}_{b}_{qb}")
                       for h in range(HPC)]
                for kt in range(S // KT):
                    k0 = t0 + kt * KT
                    sc = sc_ps.tile([128, 2 * QB], f32, tag="sc")
                    for h in range(HPC):
                        hp = h * HD
                        nc.tensor.matmul(
                            sc[:, h * QB:(h + 1) * QB],
                            kT[hp:hp + HD, k0:k0 + KT],
                            qT[hp:hp + HD, q0:q0 + QB],
                            start=True, stop=True,
                            tile_position=(hp, 0),
                        )
                    et = exp_pool.tile([128, 2 * QB], f32r, tag="expt")
                    nc.scalar.activation(et, sc, AF.Exp)
                    first, last = kt == 0, kt == S // KT - 1
                    for h in range(HPC):
                        # [V|ones] lhsT: values^T into partitions hp:hp+64,
                        # softmax denominators into the other 64 partitions
                        nc.tensor.matmul(
                            pav[h], vkm[h][:, kt, :],
                            et[:, h * QB:(h + 1) * QB],
                            start=first, stop=last,
                        )
                for h in range(HPC):
                    hp = h * HD          # partition base of this head's rows
                    op_ = (1 - h) * HD   # the other 64-partition half
                    rec = r_pool.tile([128, QB], f32, tag="rec")
                    nc.vector.reciprocal(rec[op_:op_ + HD, :], pav[h][op_:op_ + HD, :])
                    rec2 = r2_pool.tile([128, QB], f32, tag="rec2")
                    nc.sync.dma_start(
                        out=rec2[hp:hp + HD, :], in_=rec[op_:op_ + HD, :]
                    )
                    nc.vector.tensor_mul(
                        valuesT[hp:hp + HD, q0:q0 + QB],
                        pav[h][hp:hp + HD, :],
                        rec2[hp:hp + HD, :],
                    )

            # FC partial for this batch's tokens
            for tb2 in range(S // 128):
                tt = t0 + tb2 * 128
                for eb in range(D // QB):
                    fp = mm_ps.tile([128, QB], f32, tag="mm512")
                    nc.tensor.matmul(
                        fp, valuesT[:, tt:tt + 128],
                        wfc_sb[:, eb * QB:(eb + 1) * QB],
                        start=True, stop=True,
                    )
                    fo = fout_pool.tile([128, QB], f32, tag="fout")
                    nc.vector.tensor_copy(fo, fp)
                    nc.sync.dma_start(
                        out=OUT[tt:tt + 128, eb * QB:(eb + 1) * QB], in_=fo
                    )

    nc.compile()
    return nc


def _get_nc():
    global _NC_CACHE
    if _NC_CACHE is None:
        _NC_CACHE = _build()
    return _NC_CACHE


def _prep_in_maps(x, w_qkv, b_qkv, w_fc):
    xT = np.ascontiguousarray(x.reshape(T, D).T).astype(np.float32)
    in_maps = []
    for c in range(NC):
        heads = [HPC * c + i for i in range(HPC)]
        rows = {
            "q": np.concatenate([np.arange(h * 3 * HD, h * 3 * HD + HD) for h in heads]),
            "k": np.concatenate([np.arange(h * 3 * HD + HD, h * 3 * HD + 2 * HD) for h in heads]),
            "v": np.concatenate([np.arange(h * 3 * HD + 2 * HD, h * 3 * HD + 3 * HD) for h in heads]),
        }
        m = {
            "xT": xT,
            "wq": np.ascontiguousarray(w_qkv[rows["q"]].T),
            "wk": np.ascontiguousarray(w_qkv[rows["k"]].T),
            "wv": np.ascontiguousarray(w_qkv[rows["v"]].T),
            "bq": np.ascontiguousarray(b_qkv[rows["q"]][:, None]),
            "bk": np.ascontiguousarray(b_qkv[rows["k"]][:, None]),
            "bv": np.ascontiguousarray(b_qkv[rows["v"]][:, None]),
            "wfc": np.ascontiguousarray(w_fc[:, c * F:(c + 1) * F].T),
        }
        in_maps.append(m)
    return in_maps


def run_kernel(inputs, trace=False, trace_cores=None):
    x = np.asarray(inputs["x"], np.float32)
    w_qkv = np.asarray(inputs["w_qkv"], np.float32)
    b_qkv = np.asarray(inputs["b_qkv"], np.float32)
    w_fc = np.asarray(inputs["w_fc"], np.float32)
    b_fc = np.asarray(inputs["b_fc"], np.float32)

    nc = _get_nc()
    in_maps = _prep_in_maps(x, w_qkv, b_qkv, w_fc)
    res = run_bass_kernel_spmd(
        nc, in_maps, core_ids=list(range(NC)), trace=trace,
        trace_cores=trace_cores,
    )
    out = res.results[0]["out"].astype(np.float32)
    for r in res.results[1:]:
        out = out + r["out"]
    out = out + b_fc[None, :]
    return out.reshape(B, S, D), res


def kernel(**inputs):
    out, _ = run_kernel(inputs, trace=False)
    return out


# revision 12
# speedup vs baseline: 1.4580x; 1.0999x over previous
"""Multi-head attention (B=2, S=2048, D=1024, H=16) on 8 Trainium2 NeuronCores.

Sharding: tensor-parallel over heads — 2 heads per core. Each core computes
its heads' QKV projection, attention, and a partial FC output (row-slice of
the FC contraction); the host sums the 8 partials and adds the FC bias.

Per-core pipeline (all matmuls in float32r — full-rate TF32-class):
  1. QKV projection: qT/kT [128f, 4096t] transposed layouts, vT likewise.
     Score scale 1/8 and biases folded into PSUM eviction.
  2. Per (batch, head): V re-transposed to key-major [keys, 64] via PE.
  3. ScoresT [keys, q] = K^T Q per 128-key tile; exp on ACT (scores are
     bounded ~[-3, 4.5] so unsafe softmax is exact); AV accumulates
     values^T [64, q] and the softmax denominators via a parallel
     ones-matmul in the other PE column strip.
  4. Normalization: reciprocal of denominators, partition-move via DMA,
     elementwise multiply into valuesT [128f, 4096t].
  5. FC: partial[t, e] = valuesT[:, t]^T @ w_fc^T slice; DMA to DRAM.
"""
import numpy as np
from contextlib import ExitStack

import concourse.bass as bass
import concourse.tile as tile
from concourse import bacc, mybir
from concourse.bass_utils import run_bass_kernel_spmd
from concourse.masks import make_identity

B, S, D, H, HD = 2, 2048, 1024, 16, 64
T = B * S                # 4096 tokens
NC = 8                   # cores
HPC = H // NC            # heads per core
F = HPC * HD             # 128 value-features per core
KT = 128                 # key tile (contraction tile for AV)
QB = 512                 # query block (matmul free dim)
f32 = mybir.dt.float32
f32r = mybir.dt.float32r
AF = mybir.ActivationFunctionType
OP = mybir.AluOpType

_NC_CACHE = None


def _build():
    nc = bacc.Bacc("TRN2", target_bir_lowering=False, debug=False, num_devices=NC)

    XT = nc.dram_tensor("xT", [D, T], f32r, kind="ExternalInput").ap()
    WQ = nc.dram_tensor("wq", [D, F], f32r, kind="ExternalInput").ap()
    WK = nc.dram_tensor("wk", [D, F], f32r, kind="ExternalInput").ap()
    WV = nc.dram_tensor("wv", [D, F], f32r, kind="ExternalInput").ap()
    BQ = nc.dram_tensor("bq", [F, 1], f32, kind="ExternalInput").ap()
    BK = nc.dram_tensor("bk", [F, 1], f32, kind="ExternalInput").ap()
    BV = nc.dram_tensor("bv", [F, 1], f32, kind="ExternalInput").ap()
    WFC = nc.dram_tensor("wfc", [F, D], f32r, kind="ExternalInput").ap()
    OUT = nc.dram_tensor("out", [T, D], f32, kind="ExternalOutput").ap()

    with tile.TileContext(nc) as tc, ExitStack() as ctx:
        const = ctx.enter_context(tc.tile_pool(name="const", bufs=1))
        xt_pool = ctx.enter_context(tc.tile_pool(name="xt", bufs=12))
        big = ctx.enter_context(tc.tile_pool(name="big", bufs=1))
        vt_pool = ctx.enter_context(tc.tile_pool(name="vt", bufs=2))
        exp_pool = ctx.enter_context(tc.tile_pool(name="expt", bufs=4))
        r_pool = ctx.enter_context(tc.tile_pool(name="recip", bufs=2))
        r2_pool = ctx.enter_context(tc.tile_pool(name="recip2", bufs=2))
        fout_pool = ctx.enter_context(tc.tile_pool(name="fout", bufs=4))

        mm_ps = ctx.enter_context(tc.tile_pool(name="mm_ps", bufs=2, space="PSUM"))
        sc_ps = ctx.enter_context(tc.tile_pool(name="sc_ps", bufs=2, space="PSUM"))
        av_ps = ctx.enter_context(tc.tile_pool(name="av_ps", bufs=1, space="PSUM"))

        # --- constants ---
        wq_sb = const.tile([128, D // 128, F], f32r)
        nc.sync.dma_start(out=wq_sb, in_=WQ.rearrange("(t p) f -> p t f", p=128))
        wk_sb = const.tile([128, D // 128, F], f32r)
        nc.sync.dma_start(out=wk_sb, in_=WK.rearrange("(t p) f -> p t f", p=128))
        wv_sb = const.tile([128, D // 128, F], f32r)
        nc.sync.dma_start(out=wv_sb, in_=WV.rearrange("(t p) f -> p t f", p=128))
        wfc_sb = const.tile([F, D], f32r)
        nc.sync.dma_start(out=wfc_sb, in_=WFC)
        bq_sb = const.tile([F, 1], f32)
        nc.sync.dma_start(out=bq_sb, in_=BQ)
        bk_sb = const.tile([F, 1], f32)
        nc.sync.dma_start(out=bk_sb, in_=BK)
        bv_sb = const.tile([F, 1], f32)
        nc.sync.dma_start(out=bv_sb, in_=BV)

        ident = const.tile([128, 64], f32)  # I_64 stacked in both halves
        make_identity(nc, ident[0:64, :])
        make_identity(nc, ident[64:128, :])
        ones_f = const.tile([128, S // KT * HD], f32)
        nc.vector.memset(ones_f, 1.0)

        qT = big.tile([128, T], f32r)   # [Qh0(64) ; Qh1(64)] x tokens, pre-scaled 1/8
        kT = big.tile([128, T], f32r)
        vT = big.tile([128, T], f32)
        valuesT = big.tile([128, T], f32r)

        # --- phase 1: QKV projection (transposed outputs) ---
        for tb in range(T // QB):
            xts = []
            for kt in range(D // 128):
                xt = xt_pool.tile([128, QB], f32r, tag="xt")
                nc.sync.dma_start(
                    out=xt, in_=XT[kt * 128:(kt + 1) * 128, tb * QB:(tb + 1) * QB]
                )
                xts.append(xt)
            for w_sb, dst, bias_ap, scale in (
                (wq_sb, qT, bq_sb, 0.125),
                (wk_sb, kT, bk_sb, None),
                (wv_sb, vT, bv_sb, None),
            ):
                ps = mm_ps.tile([128, QB], f32, tag="mm512")
                for kt in range(D // 128):
                    nc.tensor.matmul(
                        ps, w_sb[:, kt, :], xts[kt],
                        start=(kt == 0), stop=(kt == D // 128 - 1),
                    )
                dslice = dst[:, tb * QB:(tb + 1) * QB]
                if scale is None:
                    nc.vector.tensor_scalar_add(dslice, ps, bias_ap)
                else:
                    nc.vector.tensor_scalar(
                        dslice, ps, bias_ap, scale, op0=OP.add, op1=OP.mult
                    )

        # --- phases 2-5 per batch ---
        for b in range(B):
            t0 = b * S
            # V re-transposed to key-major + ones block for the denominators:
            # head h's lhsT tile [128 keys, 128] has V in cols hp:hp+64 (so
            # values land in psum partitions hp:hp+64) and ones in the rest.
            vkm = []  # per head: [128, S//KT, 128]
            for h in range(HPC):
                hp, op_ = h * HD, (1 - h) * HD
                vk = vt_pool.tile([128, S // KT, 128], f32r, tag=f"vk{h}")
                nc.vector.tensor_copy(vk[:, :, op_:op_ + HD], ones_f)
                for kt in range(S // KT):
                    tp = mm_ps.tile([128, HD], f32, tag="mm512")
                    nc.tensor.transpose(
                        tp,
                        vT[h * HD:(h + 1) * HD, t0 + kt * KT: t0 + (kt + 1) * KT],
                        ident[h * HD:(h + 1) * HD, :],
                    )
                    nc.vector.tensor_copy(vk[:, kt, hp:hp + HD], tp)
                vkm.append(vk)

            for qb in range(S // QB):
                q0 = t0 + qb * QB
                # both heads share one [128, 2*QB] score tile so exp runs as
                # a single wide ACTIVATE; the heads' score matmuls sit in
                # disjoint PE row strips and run concurrently.
                pav = [av_ps.tile([128, QB], f32, tag=f"pav{h}",
                                  name=f"pav{h}_{b}_{qb}")
                       for h in range(HPC)]
                for kt in range(S // KT):
                    k0 = t0 + kt * KT
                    sc = sc_ps.tile([128, 2 * QB], f32, tag="sc")
                    for h in range(HPC):
                        hp = h * HD
                        nc.tensor.matmul(
                            sc[:, h * QB:(h + 1) * QB],
                            kT[hp:hp + HD, k0:k0 + KT],
                            qT[hp:hp + HD, q0:q0 + QB],
                            start=True, stop=True,
                            tile_position=(hp, 0),
                        )
                    et = exp_pool.tile([128, 2 * QB], f32r, tag="expt")
                    nc.scalar.activation(et, sc, AF.Exp)
                    first, last = kt == 0, kt == S // KT - 1
                    for h in range(HPC):
                        # [V|ones] lhsT: values^T into partitions hp:hp+64,
                        # softmax denominators into the other 64 partitions
                        nc.tensor.matmul(
                            pav[h], vkm[h][:, kt, :],
                            et[:, h * QB:(h + 1) * QB],
                            start=first, stop=last,
                        )
                # h0: values in psum parts 0:64, denoms at 64:128 (and vice
                # versa for h1). reciprocal_approx_fast only works at base
                # partition 0, so h0 stages its denominators down via DMA
                # first; h1 recips directly and stages the result up.
                den = r_pool.tile([128, QB], f32, tag="den")
                nc.vector.tensor_copy(den[64:128, :], pav[0][64:128, :])
                den2 = r_pool.tile([64, QB], f32, tag="den2")
                nc.sync.dma_start(out=den2, in_=den[64:128, :])
                rec0 = r_pool.tile([64, QB], f32, tag="rec0")
                nc.vector.reciprocal_approx_fast(out=rec0, in_=den2)
                nc.vector.tensor_mul(
                    valuesT[0:64, q0:q0 + QB], pav[0][0:64, :], rec0
                )
                rec1 = r_pool.tile([64, QB], f32, tag="rec1")
                nc.vector.reciprocal_approx_fast(out=rec1, in_=pav[1][0:64, :])
                rec1b = r2_pool.tile([128, QB], f32, tag="rec1b")
                nc.sync.dma_start(out=rec1b[64:128, :], in_=rec1)
                nc.vector.tensor_mul(
                    valuesT[64:128, q0:q0 + QB],
                    pav[1][64:128, :],
                    rec1b[64:128, :],
                )

            # FC partial for this batch's tokens
            for tb2 in range(S // 128):
                tt = t0 + tb2 * 128
                for eb in range(D // QB):
                    fp = mm_ps.tile([128, QB], f32, tag="mm512")
                    nc.tensor.matmul(
                        fp, valuesT[:, tt:tt + 128],
                        wfc_sb[:, eb * QB:(eb + 1) * QB],
                        start=True, stop=True,
                    )
                    fo = fout_pool.tile([128, QB], f32, tag="fout")
                    nc.vector.tensor_copy(fo, fp)
                    nc.sync.dma_start(
                        out=OUT[tt:tt + 128, eb * QB:(eb + 1) * QB], in_=fo
                    )

    nc.compile()
    return nc


def _get_nc():
    global _NC_CACHE
    if _NC_CACHE is None:
        _NC_CACHE = _build()
    return _NC_CACHE


def _prep_in_maps(x, w_qkv, b_qkv, w_fc):
    xT = np.ascontiguousarray(x.reshape(T, D).T).astype(np.float32)
    in_maps = []
    for c in range(NC):
        heads = [HPC * c + i for i in range(HPC)]
        rows = {
            "q": np.concatenate([np.arange(h * 3 * HD, h * 3 * HD + HD) for h in heads]),
            "k": np.concatenate([np.arange(h * 3 * HD + HD, h * 3 * HD + 2 * HD) for h in heads]),
            "v": np.concatenate([np.arange(h * 3 * HD + 2 * HD, h * 3 * HD + 3 * HD) for h in heads]),
        }
        m = {
            "xT": xT,
            "wq": np.ascontiguousarray(w_qkv[rows["q"]].T),
            "wk": np.ascontiguousarray(w_qkv[rows["k"]].T),
            "wv": np.ascontiguousarray(w_qkv[rows["v"]].T),
            "bq": np.ascontiguousarray(b_qkv[rows["q"]][:, None]),
            "bk": np.ascontiguousarray(b_qkv[rows["k"]][:, None]),
            "bv": np.ascontiguousarray(b_qkv[rows["v"]][:, None]),
            "wfc": np.ascontiguousarray(w_fc[:, c * F:(c + 1) * F].T),
        }
        in_maps.append(m)
    return in_maps


def run_kernel(inputs, trace=False, trace_cores=None):
    x = np.asarray(inputs["x"], np.float32)
    w_qkv = np.asarray(inputs["w_qkv"], np.float32)
    b_qkv = np.asarray(inputs["b_qkv"], np.float32)
    w_fc = np.asarray(inputs["w_fc"], np.float32)
    b_fc = np.asarray(inputs["b_fc"], np.float32)

    nc = _get_nc()
    in_maps = _prep_in_maps(x, w_qkv, b_qkv, w_fc)
    res = run_bass_kernel_spmd(
        nc, in_maps, core_ids=list(range(NC)), trace=trace,
        trace_cores=trace_cores,
    )
    out = res.results[0]["out"].astype(np.float32)
    for r in res.results[1:]:
        out = out + r["out"]
    out = out + b_fc[None, :]
    return out.reshape(B, S, D), res


def kernel(**inputs):
    out, _ = run_kernel(inputs, trace=False)
    return out


# revision 13
# speedup vs baseline: 1.5031x; 1.0309x over previous
"""Multi-head attention (B=2, S=2048, D=1024, H=16) on 8 Trainium2 NeuronCores.

Sharding: tensor-parallel over heads — 2 heads per core. Each core computes
its heads' QKV projection, attention, and a partial FC output (row-slice of
the FC contraction); the host sums the 8 partials and adds the FC bias.

Per-core pipeline (all matmuls in float32r — full-rate TF32-class):
  1. QKV projection: qT/kT [128f, 4096t] transposed layouts, vT likewise.
     Score scale 1/8 and biases folded into PSUM eviction.
  2. Per (batch, head): V re-transposed to key-major [keys, 64] via PE.
  3. ScoresT [keys, q] = K^T Q per 128-key tile; exp on ACT (scores are
     bounded ~[-3, 4.5] so unsafe softmax is exact); AV accumulates
     values^T [64, q] and the softmax denominators via a parallel
     ones-matmul in the other PE column strip.
  4. Normalization: reciprocal of denominators, partition-move via DMA,
     elementwise multiply into valuesT [128f, 4096t].
  5. FC: partial[t, e] = valuesT[:, t]^T @ w_fc^T slice; DMA to DRAM.
"""
import numpy as np
from contextlib import ExitStack

import concourse.bass as bass
import concourse.tile as tile
from concourse import bacc, mybir
from concourse.bass_utils import run_bass_kernel_spmd
from concourse.masks import make_identity

B, S, D, H, HD = 2, 2048, 1024, 16, 64
T = B * S                # 4096 tokens
NC = 8                   # cores
HPC = H // NC            # heads per core
F = HPC * HD             # 128 value-features per core
KT = 128                 # key tile (contraction tile for AV)
QB = 512                 # query block (matmul free dim)
f32 = mybir.dt.float32
f32r = mybir.dt.float32r
AF = mybir.ActivationFunctionType
OP = mybir.AluOpType

_NC_CACHE = None


def _build():
    nc = bacc.Bacc("TRN2", target_bir_lowering=False, debug=False, num_devices=NC)

    XT = nc.dram_tensor("xT", [D, T], f32r, kind="ExternalInput").ap()
    WQ = nc.dram_tensor("wq", [D, F], f32r, kind="ExternalInput").ap()
    WK = nc.dram_tensor("wk", [D, F], f32r, kind="ExternalInput").ap()
    WV = nc.dram_tensor("wv", [D, F], f32r, kind="ExternalInput").ap()
    BQ = nc.dram_tensor("bq", [F, 1], f32, kind="ExternalInput").ap()
    BK = nc.dram_tensor("bk", [F, 1], f32, kind="ExternalInput").ap()
    BV = nc.dram_tensor("bv", [F, 1], f32, kind="ExternalInput").ap()
    WFC = nc.dram_tensor("wfc", [F, D], f32r, kind="ExternalInput").ap()
    OUT = nc.dram_tensor("out", [T, D], f32, kind="ExternalOutput").ap()

    with tile.TileContext(nc) as tc, ExitStack() as ctx:
        const = ctx.enter_context(tc.tile_pool(name="const", bufs=1))
        xt_pool = ctx.enter_context(tc.tile_pool(name="xt", bufs=12))
        big = ctx.enter_context(tc.tile_pool(name="big", bufs=1))
        vt_pool = ctx.enter_context(tc.tile_pool(name="vt", bufs=2))
        exp_pool = ctx.enter_context(tc.tile_pool(name="expt", bufs=4))
        r_pool = ctx.enter_context(tc.tile_pool(name="recip", bufs=2))
        r2_pool = ctx.enter_context(tc.tile_pool(name="recip2", bufs=2))
        fout_pool = ctx.enter_context(tc.tile_pool(name="fout", bufs=4))

        mm_ps = ctx.enter_context(tc.tile_pool(name="mm_ps", bufs=2, space="PSUM"))
        sc_ps = ctx.enter_context(tc.tile_pool(name="sc_ps", bufs=2, space="PSUM"))
        av_ps = ctx.enter_context(tc.tile_pool(name="av_ps", bufs=1, space="PSUM"))

        # --- constants ---
        wq_sb = const.tile([128, D // 128, F], f32r)
        nc.sync.dma_start(out=wq_sb, in_=WQ.rearrange("(t p) f -> p t f", p=128))
        wk_sb = const.tile([128, D // 128, F], f32r)
        nc.sync.dma_start(out=wk_sb, in_=WK.rearrange("(t p) f -> p t f", p=128))
        wv_sb = const.tile([128, D // 128, F], f32r)
        nc.sync.dma_start(out=wv_sb, in_=WV.rearrange("(t p) f -> p t f", p=128))
        wfc_sb = const.tile([F, D], f32r)
        nc.sync.dma_start(out=wfc_sb, in_=WFC)
        bq_sb = const.tile([F, 1], f32)
        nc.sync.dma_start(out=bq_sb, in_=BQ)
        bk_sb = const.tile([F, 1], f32)
        nc.sync.dma_start(out=bk_sb, in_=BK)
        bv_sb = const.tile([F, 1], f32)
        nc.sync.dma_start(out=bv_sb, in_=BV)

        ident = const.tile([128, 64], f32)  # I_64 stacked in both halves
        make_identity(nc, ident[0:64, :])
        make_identity(nc, ident[64:128, :])
        ones_f = const.tile([128, S // KT * HD], f32)
        nc.vector.memset(ones_f, 1.0)

        qT = big.tile([128, T], f32r)   # [Qh0(64) ; Qh1(64)] x tokens, pre-scaled 1/8
        kT = big.tile([128, T], f32r)
        vT = big.tile([128, T], f32)
        valuesT = big.tile([128, T], f32r)

        # --- phase 1: QKV projection (transposed outputs) ---
        for tb in range(T // QB):
            xts = []
            for kt in range(D // 128):
                xt = xt_pool.tile([128, QB], f32r, tag="xt")
                nc.sync.dma_start(
                    out=xt, in_=XT[kt * 128:(kt + 1) * 128, tb * QB:(tb + 1) * QB]
                )
                xts.append(xt)
            for w_sb, dst, bias_ap, scale in (
                (wq_sb, qT, bq_sb, 0.125),
                (wk_sb, kT, bk_sb, None),
                (wv_sb, vT, bv_sb, None),
            ):
                ps = mm_ps.tile([128, QB], f32, tag="mm512")
                for kt in range(D // 128):
                    nc.tensor.matmul(
                        ps, w_sb[:, kt, :], xts[kt],
                        start=(kt == 0), stop=(kt == D // 128 - 1),
                    )
                dslice = dst[:, tb * QB:(tb + 1) * QB]
                if scale is None:
                    nc.vector.tensor_scalar_add(dslice, ps, bias_ap)
                else:
                    nc.vector.tensor_scalar(
                        dslice, ps, bias_ap, scale, op0=OP.add, op1=OP.mult
                    )

        # --- phases 2-5 per batch ---
        for b in range(B):
            t0 = b * S
            # V re-transposed to key-major + ones block for the denominators:
            # head h's lhsT tile [128 keys, 128] has V in cols hp:hp+64 (so
            # values land in psum partitions hp:hp+64) and ones in the rest.
            vkm = []  # per head: [128, S//KT, 128]
            for h in range(HPC):
                hp, op_ = h * HD, (1 - h) * HD
                vk = vt_pool.tile([128, S // KT, 128], f32r, tag=f"vk{h}")
                nc.vector.tensor_copy(vk[:, :, op_:op_ + HD], ones_f)
                for kt in range(S // KT):
                    tp = mm_ps.tile([128, HD], f32, tag="mm512")
                    nc.tensor.transpose(
                        tp,
                        vT[h * HD:(h + 1) * HD, t0 + kt * KT: t0 + (kt + 1) * KT],
                        ident[h * HD:(h + 1) * HD, :],
                    )
                    nc.vector.tensor_copy(vk[:, kt, hp:hp + HD], tp)
                vkm.append(vk)

            for qb in range(S // QB):
                q0 = t0 + qb * QB
                # both heads share one [128, 2*QB] score tile so exp runs as
                # a single wide ACTIVATE; the heads' score matmuls sit in
                # disjoint PE row strips and run concurrently.
                pav = [av_ps.tile([128, QB], f32, tag=f"pav{h}",
                                  name=f"pav{h}_{b}_{qb}")
                       for h in range(HPC)]
                # AV matmuls trail the score matmuls by 2 key-tiles so the
                # exp they consume is long finished when the PE reaches them
                # (a stalled wait also blocks the PE's weight-load pull-ahead)
                AV_LAG = 2
                pending = []  # (kt, et)
                NKT = S // KT

                def emit_av(kt, et):
                    for h in range(HPC):
                        # [V|ones] lhsT: values^T into partitions hp:hp+64,
                        # softmax denominators into the other 64 partitions
                        nc.tensor.matmul(
                            pav[h], vkm[h][:, kt, :],
                            et[:, h * QB:(h + 1) * QB],
                            start=(kt == 0), stop=(kt == NKT - 1),
                        )

                for kt in range(NKT):
                    k0 = t0 + kt * KT
                    sc = sc_ps.tile([128, 2 * QB], f32, tag="sc")
                    for h in range(HPC):
                        hp = h * HD
                        nc.tensor.matmul(
                            sc[:, h * QB:(h + 1) * QB],
                            kT[hp:hp + HD, k0:k0 + KT],
                            qT[hp:hp + HD, q0:q0 + QB],
                            start=True, stop=True,
                            tile_position=(hp, 0),
                        )
                    et = exp_pool.tile([128, 2 * QB], f32r, tag="expt")
                    nc.scalar.activation(et, sc, AF.Exp)
                    pending.append((kt, et))
                    if len(pending) > AV_LAG:
                        emit_av(*pending.pop(0))
                for item in pending:
                    emit_av(*item)
                # h0: values in psum parts 0:64, denoms at 64:128 (and vice
                # versa for h1). reciprocal_approx_fast only works at base
                # partition 0, so h0 stages its denominators down via DMA
                # first; h1 recips directly and stages the result up.
                den = r_pool.tile([128, QB], f32, tag="den")
                nc.vector.tensor_copy(den[64:128, :], pav[0][64:128, :])
                den2 = r_pool.tile([64, QB], f32, tag="den2")
                nc.sync.dma_start(out=den2, in_=den[64:128, :])
                rec0 = r_pool.tile([64, QB], f32, tag="rec0")
                nc.vector.reciprocal_approx_fast(out=rec0, in_=den2)
                nc.vector.tensor_mul(
                    valuesT[0:64, q0:q0 + QB], pav[0][0:64, :], rec0
                )
                rec1 = r_pool.tile([64, QB], f32, tag="rec1")
                nc.vector.reciprocal_approx_fast(out=rec1, in_=pav[1][0:64, :])
                rec1b = r2_pool.tile([128, QB], f32, tag="rec1b")
                nc.sync.dma_start(out=rec1b[64:128, :], in_=rec1)
                nc.vector.tensor_mul(
                    valuesT[64:128, q0:q0 + QB],
                    pav[1][64:128, :],
                    rec1b[64:128, :],
                )

            # FC partial for this batch's tokens
            for tb2 in range(S // 128):
                tt = t0 + tb2 * 128
                for eb in range(D // QB):
                    fp = mm_ps.tile([128, QB], f32, tag="mm512")
                    nc.tensor.matmul(
                        fp, valuesT[:, tt:tt + 128],
                        wfc_sb[:, eb * QB:(eb + 1) * QB],
                        start=True, stop=True,
                    )
                    fo = fout_pool.tile([128, QB], f32, tag="fout")
                    nc.vector.tensor_copy(fo, fp)
                    nc.sync.dma_start(
                        out=OUT[tt:tt + 128, eb * QB:(eb + 1) * QB], in_=fo
                    )

    nc.compile()
    return nc


def _get_nc():
    global _NC_CACHE
    if _NC_CACHE is None:
        _NC_CACHE = _build()
    return _NC_CACHE


def _prep_in_maps(x, w_qkv, b_qkv, w_fc):
    xT = np.ascontiguousarray(x.reshape(T, D).T).astype(np.float32)
    in_maps = []
    for c in range(NC):
        heads = [HPC * c + i for i in range(HPC)]
        rows = {
            "q": np.concatenate([np.arange(h * 3 * HD, h * 3 * HD + HD) for h in heads]),
            "k": np.concatenate([np.arange(h * 3 * HD + HD, h * 3 * HD + 2 * HD) for h in heads]),
            "v": np.concatenate([np.arange(h * 3 * HD + 2 * HD, h * 3 * HD + 3 * HD) for h in heads]),
        }
        m = {
            "xT": xT,
            "wq": np.ascontiguousarray(w_qkv[rows["q"]].T),
            "wk": np.ascontiguousarray(w_qkv[rows["k"]].T),
            "wv": np.ascontiguousarray(w_qkv[rows["v"]].T),
            "bq": np.ascontiguousarray(b_qkv[rows["q"]][:, None]),
            "bk": np.ascontiguousarray(b_qkv[rows["k"]][:, None]),
            "bv": np.ascontiguousarray(b_qkv[rows["v"]][:, None]),
            "wfc": np.ascontiguousarray(w_fc[:, c * F:(c + 1) * F].T),
        }
        in_maps.append(m)
    return in_maps


def run_kernel(inputs, trace=False, trace_cores=None):
    x = np.asarray(inputs["x"], np.float32)
    w_qkv = np.asarray(inputs["w_qkv"], np.float32)
    b_qkv = np.asarray(inputs["b_qkv"], np.float32)
    w_fc = np.asarray(inputs["w_fc"], np.float32)
    b_fc = np.asarray(inputs["b_fc"], np.float32)

    nc = _get_nc()
    in_maps = _prep_in_maps(x, w_qkv, b_qkv, w_fc)
    res = run_bass_kernel_spmd(
        nc, in_maps, core_ids=list(range(NC)), trace=trace,
        trace_cores=trace_cores,
    )
    out = res.results[0]["out"].astype(np.float32)
    for r in res.results[1:]:
        out = out + r["out"]
    out = out + b_fc[None, :]
    return out.reshape(B, S, D), res


def kernel(**inputs):
    out, _ = run_kernel(inputs, trace=False)
    return out
